# revision 1
# baseline (speedup 1.0000x reference)
"""Trainium2 Bass kernel for the NumReps masked-mean problem.

Math: each mask row is a contiguous run of ones (1..8 long). expand_window
widens it by int(0.2*len) (== 1 iff len >= 5) on each side, clamped to
[0, S-1]; the output row is the mean of reps rows over the widened window
(window length n is in {1,2,3,4} u {6,7,8,9,10}; n=6 only when the window
clamps at 0).

Strategy (per core, data-parallel over batch: 16 batches / 8 cores = 2):
  - run length via scalar-engine accumulate, position-sum via fused
    scalar_tensor_tensor passes over the mask, split into 2 S-halves so
    they overlap the mask DMA
  - ONE combined index chain for both batches on [128,2] tiles (first
    recovered exactly via the 2^23+2^22 magic rint; f32->i32 conversion
    rounds to nearest so indices must be exact); all 6 gather indices
    produced in one [128,6] tile (out-of-range +4096 trick for
    conditionally-skipped chunks)
  - split indirect-DMA gather per batch: chunks {0..3} always, {4..9}
    for n >= 6 via the +4096 out-of-bounds skip trick (measured best vs
    clamped always-execute variants); skipped landing areas pre-zeroed
    early on engines with idle slots
  - weighted windowed sum on the TensorEngine: 10 accumulating diagonal
    matmuls per batch in float32r (full rate at N=512), diag_j =
    diag((j<n)/n); gathered chunks staged through f32r tiles (rounding-op
    source required by the BIR verifier), casts split DVE/ACT (gpsimd is
    ~8x slower at casts - never cast there)
  - PSUM -> SBUF copies (DVE + ACT) cast to bf16 (tolerance is 2e-2;
    bf16 keeps rel err ~4e-4 while halving the output DMA), store
"""

import numpy as np

B, M, S, D = 16, 128, 2048, 1024
NCORES = 8
BPC = B // NCORES  # batches per core
WMAX = 10  # max expanded window length

# gather plan: (chunk_start, n_chunks, n-threshold or None=always)
GATHER_PLAN = [(0, 4, None), (4, 4, 4.5), (8, 2, 8.5)]
ZCHUNK = 4  # first chunk whose landing area needs pre-zeroing
RINT_MAGIC = 12582912.0  # 2^23 + 2^22

_cache = {}


def _build_nc():
    import concourse.bacc as bacc
    import concourse.bass as bass
    import concourse.mybir as mybir
    from concourse import tile

    f32 = mybir.dt.float32
    f32r = mybir.dt.float32r
    bf16 = mybir.dt.bfloat16
    i16 = mybir.dt.int16
    i32 = mybir.dt.int32
    Alu = mybir.AluOpType
    Act = mybir.ActivationFunctionType

    nc = bacc.Bacc("TRN2", target_bir_lowering=False, debug=False)

    mask = nc.dram_tensor("mask", [BPC, M, S], f32, kind="ExternalInput")
    reps = [
        nc.dram_tensor(f"reps{b}", [S, D], f32, kind="ExternalInput")
        for b in range(BPC)
    ]
    out = nc.dram_tensor("out", [BPC, M, D], bf16, kind="ExternalOutput")

    K = len(GATHER_PLAN)
    BK = BPC * K
    H = S // 2

    # inline consts
    thr_np = np.array([(-1e9 if th is None else th)
                       for (_, _, th) in GATHER_PLAN], dtype=np.float32)
    off_np = np.array([cs + 4096.0 for (cs, _, _) in GATHER_PLAN],
                      dtype=np.float32)
    tc_np = np.broadcast_to(
        np.concatenate([thr_np, off_np])[None, :], (M, 2 * K))
    tc_const = nc.inline_tensor(np.ascontiguousarray(tc_np), name="tc_const")

    with tile.TileContext(nc) as tc:
        with (
            tc.tile_pool(name="const", bufs=1) as cpool,
            tc.tile_pool(name="big", bufs=2) as big,
            tc.tile_pool(name="small", bufs=2) as small,
            tc.tile_pool(name="psum", bufs=2, space="PSUM") as psum,
        ):
            # DMA issue order puts the first-needed bytes first; iota is
            # generated on gpsimd (idle at t=0) to keep DMA for the masks
            mts = []
            for b in range(BPC):
                mt = big.tile([M, S], f32, tag=f"mask{b}", name=f"mask{b}")
                mts.append(mt)
            nc.sync.dma_start(mts[0][:, :H], mask[0][:, :H])
            nc.sync.dma_start(mts[0][:, H:], mask[0][:, H:])
            nc.sync.dma_start(mts[1][:, :H], mask[1][:, :H])
            nc.sync.dma_start(mts[1][:, H:], mask[1][:, H:])
            toc = cpool.tile([M, 2 * K], f32)
            nc.sync.dma_start(toc[:], tc_const[:])
            thr_t = toc[:, :K]
            off_t = toc[:, K:]

            iota_f = cpool.tile([M, S], i16)
            nc.gpsimd.iota(iota_f[:], pattern=[[1, S]], base=0,
                           channel_multiplier=0)
            ident = cpool.tile([M, M], f32)
            nc.gpsimd.memset(ident[:], 1.0)
            nc.gpsimd.affine_select(
                out=ident[:], in_=ident[:], compare_op=Alu.is_equal,
                fill=0.0, base=0, pattern=[[-1, M]], channel_multiplier=1,
            )

            # explicit gather tiles (one per batch); skipped tail areas
            # pre-zeroed once, spread thinly across engines' idle slots
            gts = [
                cpool.tile([M, WMAX * D], f32, tag=f"gt{b}", name=f"gt{b}")
                for b in range(BPC)
            ]
            nc.vector.memset(gts[0][:, ZCHUNK * D:8 * D], 0.0)
            nc.gpsimd.memset(gts[1][:, ZCHUNK * D:8 * D], 0.0)
            nc.gpsimd.memset(gts[0][:, 8 * D:], 0.0)
            nc.scalar.memzero(gts[1][:, 8 * D:])

            # len on ACT (accum of mask), possum partial on DVE; elementwise
            # outs land in stride-0 one-column sinks. lh/ah columns (b, h)
            lsink = small.tile([M, 1], f32, tag="lsink")
            lh = small.tile([M, 4], f32, tag="lh")
            lsink_ap = bass.AP(
                lsink[:].tensor, lsink[:].offset, [lsink[:].ap[0], [0, H]]
            )
            ssink = small.tile([M, 1], f32, tag="ssink")
            ah = small.tile([M, 4], f32, tag="ah")
            ssink_ap = bass.AP(
                ssink[:].tensor, ssink[:].offset, [ssink[:].ap[0], [0, H]]
            )
            for b in range(BPC):
                for h in range(2):
                    nc.scalar.activation(
                        out=lsink_ap, in_=mts[b][:, h * H:(h + 1) * H],
                        func=Act.Identity,
                        accum_out=lh[:, 2 * b + h:2 * b + h + 1],
                    )
            for h in range(2):
                nc.vector.scalar_tensor_tensor(
                    out=ssink_ap, in0=iota_f[:, h * H:(h + 1) * H],
                    scalar=0.0, in1=mts[0][:, h * H:(h + 1) * H],
                    op0=Alu.add, op1=Alu.mult,
                    accum_out=ah[:, h:h + 1],
                )

            # per-batch index chain + gathers, batch-major so batch 0's
            # whole pipeline (gather-cast-matmul-store) completes early and
            # overlaps batch 1's gather
            ns2 = small.tile([M, BPC], f32, tag="ns2")
            n2 = small.tile([M, BPC], f32, tag="n2")
            inv2 = small.tile([M, BPC], f32, tag="inv2")
            nds = []
            zt = small.tile([M, 1], f32, tag="zt")
            for b in range(BPC):
              with tc.high_priority():
                  if b == 1:
                      # possum passes for batch 1, data-gated on batch 0's
                      # index chain (zt == 0) so they schedule after it
                      for h in range(2):
                          nc.vector.scalar_tensor_tensor(
                              out=ssink_ap,
                              in0=iota_f[:, h * H:(h + 1) * H],
                              scalar=zt[:, :1],
                              in1=mts[1][:, h * H:(h + 1) * H],
                              op0=Alu.add, op1=Alu.mult,
                              accum_out=ah[:, 2 + h:3 + h],
                          )
                  lenf = small.tile([M, 1], f32, tag=f"lenf{b}",
                                    name=f"lenf{b}")
                  nc.vector.tensor_tensor(
                      out=lenf[:], in0=lh[:, 2 * b:2 * b + 1],
                      in1=lh[:, 2 * b + 1:2 * b + 2], op=Alu.add)
                  psm = small.tile([M, 1], f32, tag=f"psm{b}", name=f"psm{b}")
                  nc.vector.tensor_tensor(
                      out=psm[:], in0=ah[:, 2 * b:2 * b + 1],
                      in1=ah[:, 2 * b + 1:2 * b + 2], op=Alu.add)
                  rl = small.tile([M, 1], f32, tag=f"rl{b}", name=f"rl{b}")
                  nc.vector.reciprocal(rl[:], lenf[:])
                  # hl = (len-1)/2 (can be half-integer: keep it away from
                  # the magic constant, whose ulp is 1.0)
                  hl = small.tile([M, 1], f32, tag=f"hl{b}", name=f"hl{b}")
                  nc.vector.tensor_scalar(
                      out=hl[:], in0=lenf[:], scalar1=0.5,
                      scalar2=-0.5, op0=Alu.mult, op1=Alu.add)
                  first = small.tile([M, 1], f32, tag=f"first{b}",
                                     name=f"first{b}")
                  nc.vector.tensor_scalar(
                      out=first[:], in0=psm[:], scalar1=rl[:, :1],
                      scalar2=hl[:, :1], op0=Alu.mult, op1=Alu.subtract)
                  nc.vector.tensor_scalar(
                      out=first[:], in0=first[:], scalar1=RINT_MAGIC,
                      scalar2=-RINT_MAGIC, op0=Alu.add, op1=Alu.add)
                  e = small.tile([M, 1], f32, tag=f"e{b}", name=f"e{b}")
                  nc.vector.tensor_scalar(
                      out=e[:], in0=lenf[:], scalar1=4.5, scalar2=None,
                      op0=Alu.is_ge)
                  ns = ns2[:, b:b + 1]
                  nc.vector.tensor_scalar(
                      out=ns, in0=first[:], scalar1=e[:, :1],
                      scalar2=0.0, op0=Alu.subtract, op1=Alu.max)
                  # t = len + e - 1; ne = min(first + t, S-1)
                  t_ = small.tile([M, 1], f32, tag=f"t{b}", name=f"t{b}")
                  nc.vector.tensor_scalar(
                      out=t_[:], in0=lenf[:], scalar1=e[:, :1],
                      scalar2=-1.0, op0=Alu.add, op1=Alu.add)
                  ne = small.tile([M, 1], f32, tag=f"ne{b}", name=f"ne{b}")
                  nc.vector.tensor_scalar(
                      out=ne[:], in0=first[:], scalar1=t_[:, :1],
                      scalar2=float(S - 1), op0=Alu.add, op1=Alu.min)
                  n_ = n2[:, b:b + 1]
                  nc.vector.scalar_tensor_tensor(
                      out=n_, in0=ne[:], scalar=1.0, in1=ns,
                      op0=Alu.add, op1=Alu.subtract)

                  # gather indices for this batch in one [M, K] tile:
                  # idx = cv*(-4096) + (ns + start + 4096); cv=0 pushes the
                  # index out of bounds so the descriptor is skipped
                  cvb = small.tile([M, K], f32, tag=f"cv{b}", name=f"cv{b}")
                  nc.vector.tensor_tensor(
                      out=cvb[:], in0=n_.to_broadcast([M, K]), in1=thr_t,
                      op=Alu.is_ge)
                  qvb = small.tile([M, K], f32, tag=f"qv{b}", name=f"qv{b}")
                  nc.vector.tensor_scalar(
                      out=qvb[:], in0=off_t, scalar1=ns, scalar2=None,
                      op0=Alu.add)
                  idxf = small.tile([M, K], f32, tag=f"idxf{b}",
                                    name=f"idxf{b}")
                  nc.vector.scalar_tensor_tensor(
                      out=idxf[:], in0=cvb[:], scalar=-4096.0, in1=qvb[:],
                      op0=Alu.mult, op1=Alu.add)
                  idxi = small.tile([M, K], i32, tag=f"idxi{b}",
                                    name=f"idxi{b}")
                  nc.vector.tensor_copy(idxi[:], idxf[:])

                  for k, (cs, nch, th) in enumerate(GATHER_PLAN):
                      kw = {}
                      if th is not None:
                          kw = dict(bounds_check=S - 1, oob_is_err=False)
                      nc.gpsimd.indirect_dma_start(
                          out=gts[b][:, cs * D:(cs + nch) * D],
                          out_offset=None,
                          in_=reps[b][:],
                          in_offset=bass.IndirectOffsetOnAxis(
                              ap=idxi[:, k:k + 1], axis=0),
                          **kw,
                      )
                  # nd = n + 0*idxi: a real data dependency that keeps the
                  # scheduler from slotting the big diag build ahead of the
                  # tiny idx ops (observed: priority alone does not)
                  nd_ = n2[:, b:b + 1]
                  nd = small.tile([M, 1], f32, tag=f"nd{b}", name=f"nd{b}")
                  nc.vector.scalar_tensor_tensor(
                      out=nd[:], in0=idxi[:, :1], scalar=0.0, in1=nd_,
                      op0=Alu.mult, op1=Alu.add)
                  nc.vector.reciprocal(inv2[:, b:b + 1], nd[:])
                  nds.append(nd)
                  if b == 0:
                      nc.vector.tensor_scalar(
                          out=zt[:], in0=idxf[:, :1], scalar1=0.0,
                          scalar2=None, op0=Alu.mult)

            # per-batch: weights/diag, staged casts + matmuls, store.
            # fp32r rhs must be produced by a rounding op (the verifier keys
            # on the memory location, so the DMA-written gather tile can't
            # feed the PE directly); casts mostly DVE, one pair on ACT
            cast_eng = {0: "v", 1: "v", 2: "a", 3: "v", 4: "v"}
            warm_ps = psum.tile([M, 512], f32, tag="warm", bufs=1)
            for b in range(BPC):
                gt = gts[b]
                w = small.tile([M, WMAX], f32, tag="w")
                nc.vector.tensor_scalar(
                    out=w[:], in0=iota_f[:, :WMAX], scalar1=nds[b][:, :1],
                    scalar2=inv2[:, b:b + 1], op0=Alu.is_lt, op1=Alu.mult)
                diag = big.tile([M, WMAX * M], f32r, tag="diag")
                nc.vector.tensor_tensor(
                    out=diag[:].rearrange("p (j q) -> p j q", j=WMAX),
                    in0=ident[:].unsqueeze(1).to_broadcast([M, WMAX, M]),
                    in1=w[:].unsqueeze(-1).to_broadcast([M, WMAX, M]),
                    op=Alu.mult,
                )

                osum = big.tile([M, D], bf16, tag="osum")
                ps0 = psum.tile([M, 512], f32, tag="ps0")
                ps1 = psum.tile([M, 512], f32, tag="ps1")
                jstart, jstop = 0, WMAX - 1
                for h in range(WMAX // 2):
                    gtr = big.tile([M, 2 * D], f32r, tag="gtr", bufs=5,
                                   name=f"gtr_{b}_{h}")
                    src = gt[:, 2 * h * D:(2 * h + 2) * D]
                    if cast_eng[h] == "v":
                        nc.vector.tensor_copy(gtr[:], src)
                    else:
                        nc.scalar.copy(gtr[:], src)
                    for j in (2 * h, 2 * h + 1):
                        dj = diag[:, j * M:(j + 1) * M]
                        seg = gtr[:, (j % 2) * D:(j % 2 + 1) * D]
                        nc.tensor.matmul(
                            ps0[:], lhsT=dj, rhs=seg[:, :512],
                            start=(j == jstart), stop=(j == jstop),
                        )
                        nc.tensor.matmul(
                            ps1[:], lhsT=dj, rhs=seg[:, 512:],
                            start=(j == jstart), stop=(j == jstop),
                        )
                nc.vector.tensor_copy(osum[:, :512], ps0[:])
                nc.sync.dma_start(out[b][:, :512], osum[:, :512])
                nc.scalar.copy(osum[:, 512:], ps1[:])
                nc.sync.dma_start(out[b][:, 512:], osum[:, 512:])
                if b == 0:
                    # keep the PE p-state warm across the inter-batch gap:
                    # dummies gated on batch 0's osum (non-rotating slot, so
                    # no pool WAR - reading a pooled gtr here cost +7us)
                    for wi in range(12):
                        nc.tensor.matmul(
                            warm_ps[:], lhsT=osum[:, :M],
                            rhs=osum[:, :512], start=True, stop=True,
                        )

    nc.finalize()
    return nc


def _get_nc():
    if "nc" not in _cache:
        _cache["nc"] = _build_nc()
    return _cache["nc"]


def _shard_inputs(number_mask, reps):
    in_maps = []
    for c in range(NCORES):
        m = {"mask": np.ascontiguousarray(number_mask[c * BPC:(c + 1) * BPC])}
        for b in range(BPC):
            m[f"reps{b}"] = np.ascontiguousarray(reps[c * BPC + b])
        in_maps.append(m)
    return in_maps


def _install_ntff_hook():
    """The image's antenv lacks axon_hooks; synthesize it so trace=True
    (NTFF profiling) works through run_bass_kernel_spmd."""
    import sys
    import types

    try:
        from antenv.axon_hooks import get_axon_ntff_profile_hook  # noqa: F401
        return
    except ImportError:
        pass
    from trn_agent_boot.trn_boot import _ntff_profile_via_ctypes

    mod = types.ModuleType("antenv.axon_hooks")
    _hook = [_ntff_profile_via_ctypes("/opt/axon/libaxon_pjrt.so")]
    mod.get_axon_ntff_profile_hook = lambda: _hook[0]
    mod.set_axon_ntff_profile_hook = lambda h: _hook.__setitem__(0, h)
    sys.modules["antenv.axon_hooks"] = mod
    import antenv

    antenv.axon_hooks = mod


def _run(number_mask, reps, trace=False):
    from concourse.bass_utils import run_bass_kernel_spmd

    if trace:
        _install_ntff_hook()
    nc = _get_nc()
    in_maps = _shard_inputs(number_mask, reps)
    res = run_bass_kernel_spmd(
        nc, in_maps, core_ids=list(range(NCORES)), trace=trace
    )
    outs = np.stack([np.asarray(r["out"]).astype(np.float32)
                     for r in res.results], axis=0)
    return outs.reshape(B, M, D), res


def kernel(**inputs):
    out, _ = _run(inputs["number_mask"], inputs["reps"], trace=False)
    return out

